# revision 36
# baseline (speedup 1.0000x reference)
"""Trainium2 Bass kernel for nn_CombinedGNN (gnn_message_passing).

Strategy (8 NeuronCores, node/row parallel, zero collectives):
  - masks[1] in the reference is identically zero, so only mask0 = adj/rowdeg
    matters.  All T=12 timesteps' aggregations batch into ONE matmul
    adj^T-shard @ X with X = data rearranged to [N, 96].
  - adj ships as fp8e4 (0/1 exactly representable -> exact, half the DMA
    bytes); row normalization uses a host-computed reciprocal-degree tile
    (bf16, replicated to 96 partitions) applied in one DVE op.  Chunks are
    DRAM-contiguous and alternate across both HWDGE queues.
  - The sequential t-chain runs both node-halves in lockstep with state
    packed at 32-partition bases (z_A@0:8, z_B@32:40 in one PSUM bank;
    s_A@64:72, s_B@96:104 in another).  Per step the serial cycle is
    C2 [ACT: prev=relu(s), shifted 64:104->0:40] -> z matmuls -> C1 [DVE:
    h2=relu(z)+pos] -> s matmuls; the per-step matmuls land on disjoint
    32x32 PE tiles via explicit tile_position so A/B pairs run
    concurrently.  All matmul OPERANDS stay at partition bases 0/32
    (base-64/96 operands hang this walrus); only outputs use 64/96.
    At t=0 the agg operand is read straight out of the transition result
    (partition base 0), skipping the first scatter round-trip.
  - Dummy warmup matmuls bridge the PE from init to the first data so the
    HAM clock holds 2.4GHz through phase 1.  Output is bf16, relu'd on
    DVE per half, each half's store overlapping the other's compute.
"""

import numpy as np
import ml_dtypes

import concourse.bass as bass
import concourse.mybir as mybir
import concourse.bass_utils as bass_utils
from concourse.tile import TileContext
from concourse.vector_clock import ScopedClock
from contextlib import contextmanager


@contextmanager
def _lean_drain():
    """Skip end-of-kernel semaphore clears (one-shot NEFF; every
    run_bass_kernel_spmd call reloads the NEFF, which re-zeros sems)."""
    orig = TileContext._drain_and_barrier

    def patched(self, tick_clock, wait_clock):
        nc = self.nc
        drain_inst = nc.sync.drain()
        wait_clock.add_sem_waits(
            drain_inst.ins, ScopedClock({None: tick_clock.global_clock}))
        nc.all_engine_barrier()
        popped = nc._tile_sem_poison_stack.pop()
        assert popped is self._sem_poison
        nc.all_engine_barrier()

    TileContext._drain_and_barrier = patched
    try:
        yield
    finally:
        TileContext._drain_and_barrier = orig

# problem constants (hardcoded per harness contract)
N, T, DAY, L = 5000, 12, 8, 2
F = DAY - 1
DIM = T * DAY  # 96
NCORES = 8
NPC = N // NCORES        # 625 nodes per core
NP = 640                 # padded nodes per core
NH = NP // 2             # 320, node half processed per psum chunk
KT = 128                 # contraction tile (partitions; K padded to 5120)
NK = 5120                # padded contraction size
NKT = NK // KT           # 40
KG = 10                  # k-tiles per DMA chunk
NG = NKT // KG           # 4 chunks per half

F32 = mybir.dt.float32
BF16 = mybir.dt.bfloat16
FP8 = mybir.dt.float8e4
BF16_NP = ml_dtypes.bfloat16
FP8_NP = ml_dtypes.float8_e4m3fn

_MAXW = 1

NWARM = 13               # dummy warmup matmuls (N=512 each)


def split_multi_waits(nc):
    """Walrus in this container rejects instructions with >~2 sync waits.
    Hoist extra waits onto preceding single-wait NoOps on the same engine."""
    f = nc.m.functions[0]
    for bb in list(f.blocks):
        new, ctr = [], 0
        for inst in bb.instructions:
            si = inst.sync_info
            waits = list(si.on_wait) if (si and si.on_wait) else []
            if len(waits) > _MAXW:
                head, keep = waits[:-_MAXW], waits[-_MAXW:]
                for i in range(0, len(head), _MAXW):
                    nop = mybir.InstNoOp(
                        name=f"{inst.name}-wsplit{ctr}", engine=inst.engine,
                        ins=[], outs=[],
                        sync_info=mybir.SyncInfo(on_wait=head[i:i + _MAXW],
                                                 on_update=[]),
                    )
                    ctr += 1
                    new.append(nop)
                inst.sync_info = mybir.SyncInfo(
                    on_wait=keep,
                    on_update=list(si.on_update) if si.on_update else [])
            new.append(inst)
        bb.instructions = new


def build_nc():
    with _lean_drain():
        return _build_nc_inner()


def _build_nc_inner():
    nc = bass.Bass()
    # DRAM-contiguous chunk layouts
    a_d = nc.dram_tensor("a", [2, NG, KT, KG, NH], FP8, kind="ExternalInput")
    xe_d = nc.dram_tensor("xe", [2, KT, NKT // 2, DIM], FP8,
                          kind="ExternalInput")
    rb_d = nc.dram_tensor("rb", [DIM, NP], BF16, kind="ExternalInput")
    dt_d = nc.dram_tensor("dt", [2, 8, T * NH], BF16, kind="ExternalInput")
    pt_d = nc.dram_tensor("pt", [2, 8, T * NH], BF16, kind="ExternalInput")
    # chain weights, pre-placed at their partition homes (see chain layout)
    w1p_d = nc.dram_tensor("w1p", [40, T * 8], BF16, kind="ExternalInput")
    w1ra_d = nc.dram_tensor("w1ra", [48, T * 8], BF16, kind="ExternalInput")
    w1x_d = nc.dram_tensor("w1x", [8, 8], BF16, kind="ExternalInput")
    w2s_d = nc.dram_tensor("w2s", [40, T * 8], BF16, kind="ExternalInput")
    wcf_d = nc.dram_tensor("wcf", [40, T * DIM], BF16, kind="ExternalInput")
    out_d = nc.dram_tensor("out", [DIM, NP], BF16, kind="ExternalOutput")

    with TileContext(nc) as tc:
        with (
            tc.tile_pool(name="const", bufs=1) as cpool,
            tc.tile_pool(name="adma", bufs=5) as apool,
            tc.tile_pool(name="hpool", bufs=6) as hpool,
            tc.tile_pool(name="pagg", bufs=2, space="PSUM") as pagg,
            tc.tile_pool(name="pst", bufs=1, space="PSUM") as pst,
            tc.tile_pool(name="pwarm", bufs=1, space="PSUM") as pwarm,
            tc.tile_pool(name="pcm", bufs=1, space="PSUM") as pcm,
        ):
            AX = mybir.AluOpType.max
            AA = mybir.AluOpType.add
            AM = mybir.AluOpType.mult
            RELU = mybir.ActivationFunctionType.Relu

            # --- tiles
            wl_t = cpool.tile([KT, 8], BF16)        # zeros (warmup lhsT)
            wr_t = cpool.tile([KT, 512], BF16)      # zeros (warmup rhs)
            xe_t = cpool.tile([KT, NKT, DIM], FP8)
            posz_t = cpool.tile([40, T * NH], BF16)
            dagra_t = cpool.tile([48, T * NH], BF16)
            w1p_t = cpool.tile([40, T * 8], BF16)
            w1ra_t = cpool.tile([48, T * 8], BF16)
            w1x_t = cpool.tile([8, 8], BF16)
            w2s_t = cpool.tile([40, T * 8], BF16)
            wcf_t = cpool.tile([40, T * DIM], BF16)
            rb_t = cpool.tile([DIM, NP], BF16)
            aggs_h = [cpool.tile([DIM, NH], BF16, name=f"aggs{h}")
                      for h in range(2)]
            outt_t = cpool.tile([DIM, NP], BF16)

            st_z = pst.tile([40, NH], F32)    # z_A 0:8 | z_B 32:40
            st_s = pst.tile([104, NH], F32)   # s_A 64:72 | s_B 96:104
            warm_p = pwarm.tile([8, 512], F32)
            aggp_t = [pagg.tile([DIM, NH], F32, tag="aggp", name=f"aggp{h}")
                      for h in range(2)]
            pcomb_t = pcm.tile([DIM, 2, 512], F32)  # one bank per half

            # --- memsets (no deps; run immediately)
            nc.vector.memset(wl_t, 0.0)
            nc.vector.memset(wr_t, 0.0)

            # --- warmup + s-row zero-init (all inputs are local zeros)
            for i in range(NWARM):
                nc.tensor.matmul(warm_p, wl_t, wr_t, start=True, stop=True)
            nc.tensor.matmul(st_s[64:72, :], wl_t[0:8, :], wr_t[0:8, 0:NH],
                             start=True, stop=False, skip_group_check=True,
                             tile_position=(0, 64))
            nc.tensor.matmul(st_s[96:104, :], wl_t[0:8, :], wr_t[0:8, 0:NH],
                             start=True, stop=False, skip_group_check=True,
                             tile_position=(0, 96))

            # --- input DMAs; pos first (feeds wf-pos warm matmuls), then
            # adj chunks alternating across both HWDGE queues
            a_tiles = {}

            def a_dma(eng, h, g):
                a_t = apool.tile([KT, KG, NH], FP8, tag="a", name=f"a{h}{g}")
                eng.dma_start(out=a_t, in_=a_d[h, g, :, :, :])
                a_tiles[(h, g)] = a_t

            nc.sync.dma_start(out=xe_t[:, 0:20, :], in_=xe_d[0])
            a_dma(nc.scalar, 0, 0)
            a_dma(nc.sync, 0, 1)
            nc.scalar.dma_start(out=xe_t[:, 20:40, :], in_=xe_d[1])
            a_dma(nc.sync, 0, 2)
            a_dma(nc.scalar, 0, 3)
            a_dma(nc.sync, 1, 0)
            a_dma(nc.scalar, 1, 1)
            a_dma(nc.sync, 1, 2)
            a_dma(nc.scalar, 1, 3)

            # small tensors ride the gpsimd SWDGE queue (rb/weights first)
            nc.gpsimd.dma_start(out=rb_t, in_=rb_d[:, :])
            nc.gpsimd.dma_start(out=w1p_t, in_=w1p_d[:, :])
            nc.gpsimd.dma_start(out=w1ra_t, in_=w1ra_d[:, :])
            nc.gpsimd.dma_start(out=w1x_t, in_=w1x_d[:, :])
            nc.gpsimd.dma_start(out=w2s_t, in_=w2s_d[:, :])
            nc.gpsimd.dma_start(out=wcf_t, in_=wcf_d[:, :])

            def ns(t):
                return slice(t * NH, (t + 1) * NH)

            def w8(t):
                return slice(t * 8, t * 8 + 8)

            def wc(t):
                return slice(t * DIM, (t + 1) * DIM)

            # phase 1 chunk: 10 matmuls aggp[96, NH] += xe_k^T @ a_k
            def phase1_chunk(h, g):
                a_t = a_tiles[(h, g)]
                for j in range(KG):
                    k = g * KG + j
                    nc.tensor.matmul(aggp_t[h], xe_t[:, k, :], a_t[:, j, :],
                                     start=(k == 0), stop=(k == NKT - 1))

            # transition: scale by reciprocal degree (one DVE op), scatter
            # scaled agg rows (8t+d) -> dagra agg rows, slab t
            def transition(h):
                cs = slice(h * NH, (h + 1) * NH)
                rbase = 32 * h
                nc.vector.scalar_tensor_tensor(
                    aggs_h[h], aggp_t[h], 0.0, rb_t[:, cs],
                    op0=AA, op1=AM)
                for t in range(T):
                    eng = nc.gpsimd if h == 0 else nc.sync
                    eng.dma_start(
                        out=dagra_t[rbase + 8:rbase + 16, ns(t)],
                        in_=aggs_h[h][t * 8:(t + 1) * 8, :])
                    if h == 0 and t == 1:
                        # dt/pos ride q0 after the adj stream has drained
                        nc.gpsimd.dma_start(out=dagra_t[0:8, :], in_=dt_d[0])
                        nc.gpsimd.dma_start(out=dagra_t[32:40, :],
                                            in_=dt_d[1])
                        nc.gpsimd.dma_start(out=posz_t[0:8, :], in_=pt_d[0])
                        nc.gpsimd.dma_start(out=posz_t[32:40, :],
                                            in_=pt_d[1])

            # --- the lockstep chain ---------------------------------------
            # state bank rows: z_A 0:8, z_B 32:40, s_A 64:72, s_B 96:104.
            # All matmul OPERANDS stay at partition bases 0/32 (base-64/96
            # operands hang this walrus); only matmul OUTPUTS use 64/96.
            # Per t: C2 [ACT] prev(t) = relu(s(t-1)), shifted 64:104->0:40;
            #        C1 [DVE] h2(t) = relu(z(t)) + pos(t), rows 0:40.
            def chain_step(t):
                # C2: prev(t) = relu(s(t-1)), ACT, partition shift 64->0.
                # Runs before this step's s matmuls touch the s rows.
                if t > 0:
                    hstp = hpool.tile([40, NH], BF16, tag="hstp",
                                      name=f"hstp{t}")
                    hstps.append(hstp)
                    nc.scalar.activation(hstp, st_s[64:104, :], RELU)
                # za: z_h = w1ra(t)^T @ [raw; agg](t)   (resets z rows)
                if t == 0:
                    nc.tensor.matmul(st_z[0:8, :], w1ra_t[0:8, 0:8],
                                     dagra_t[0:8, 0:NH],
                                     start=True, stop=False,
                                     skip_group_check=True,
                                     tile_position=(0, 0))
                    nc.tensor.matmul(st_z[0:8, :], w1x_t,
                                     aggs_h[0][0:8, :],
                                     start=False, stop=True,
                                     skip_group_check=True,
                                     tile_position=(0, 0))
                    nc.tensor.matmul(st_z[32:40, :], w1ra_t[32:40, 0:8],
                                     dagra_t[32:40, 0:NH],
                                     start=True, stop=False,
                                     skip_group_check=True,
                                     tile_position=(32, 32))
                    nc.tensor.matmul(st_z[32:40, :], w1x_t,
                                     aggs_h[1][0:8, :],
                                     start=False, stop=True,
                                     skip_group_check=True,
                                     tile_position=(0, 32))
                else:
                    nc.tensor.matmul(st_z[0:8, :], w1ra_t[0:16, w8(t)],
                                     dagra_t[0:16, ns(t)],
                                     start=True, stop=False,
                                     skip_group_check=True,
                                     tile_position=(0, 0))
                    nc.tensor.matmul(st_z[32:40, :], w1ra_t[32:48, w8(t)],
                                     dagra_t[32:48, ns(t)],
                                     start=True, stop=False,
                                     skip_group_check=True,
                                     tile_position=(32, 32))
                # zb: z_h += w1p(t)^T @ prev_h(t)
                if t > 0:
                    hp = hstps[t - 1]
                    nc.tensor.matmul(st_z[0:8, :], w1p_t[0:8, w8(t)],
                                     hp[0:8, :],
                                     start=False, stop=True,
                                     skip_group_check=True,
                                     tile_position=(0, 0))
                    nc.tensor.matmul(st_z[32:40, :], w1p_t[32:40, w8(t)],
                                     hp[32:40, :],
                                     start=False, stop=True,
                                     skip_group_check=True,
                                     tile_position=(32, 32))
                # C1: h2(t) = relu(z(t)) + pos(t), DVE, rows 0:40
                hst = hpool.tile([40, NH], BF16, tag="hst", name=f"hst{t}")
                hsts.append(hst)
                nc.vector.scalar_tensor_tensor(
                    hst, st_z[0:40, :], 0.0, posz_t[:, ns(t)], op0=AX, op1=AA)
                # s_h += w2s(t)^T @ h2_h(t)
                nc.tensor.matmul(st_s[64:72, :], w2s_t[0:8, w8(t)],
                                 hst[0:8, :],
                                 start=False, stop=(t == T - 1),
                                 skip_group_check=True, tile_position=(0, 64))
                nc.tensor.matmul(st_s[96:104, :], w2s_t[32:40, w8(t)],
                                 hst[32:40, :],
                                 start=False, stop=(t == T - 1),
                                 skip_group_check=True,
                                 tile_position=(32, 96))
                # wf: pcomb_h += wcf(t)^T @ h2_h(t)  (output accumulation,
                # off the critical path)
                for h in range(2):
                    pb = 32 * h
                    nc.tensor.matmul(pcomb_t[:, h, 0:NH], wcf_t[pb:pb + 8, wc(t)],
                                     hst[pb:pb + 8, :],
                                     start=(t == 0), stop=(t == T - 1),
                                     skip_group_check=True,
                                     tile_position=(pb, 0))

            def final():
                nc.vector.tensor_scalar_max(
                    outt_t[:, 0:NH], pcomb_t[:, 0, 0:NH], 0.0)
                nc.sync.dma_start(out=out_d[:, 0:NH], in_=outt_t[:, 0:NH])
                nc.vector.tensor_scalar_max(
                    outt_t[:, NH:NP], pcomb_t[:, 1, 0:NH], 0.0)
                nc.scalar.dma_start(out=out_d[:, NH:NP],
                                    in_=outt_t[:, NH:NP])

            # --- emission order == PE FIFO order.
            hsts, hstps = [], []
            for g in range(NG):
                phase1_chunk(0, g)
            transition(0)
            for g in range(NG):
                phase1_chunk(1, g)
            transition(1)
            for t in range(T):
                chain_step(t)
            final()

    split_multi_waits(nc)
    return nc


def prep_in_maps(adj, data, pos, his_W, cur_W, his_weight, cur_weight,
                 final_weight):
    adj = np.asarray(adj, dtype=np.float32)
    data = np.asarray(data, dtype=np.float32)
    pos = np.asarray(pos, dtype=np.float32)
    his_W = np.asarray(his_W, dtype=np.float32)
    cur_W = np.asarray(cur_W, dtype=np.float32)
    his_weight = np.asarray(his_weight, dtype=np.float32)
    cur_weight = np.asarray(cur_weight, dtype=np.float32)
    final_weight = np.asarray(final_weight, dtype=np.float32)

    # X = data rearranged [N, 96] (col = t*8+d); contraction dim zero-padded
    X = np.ascontiguousarray(data.transpose(1, 0, 2).reshape(N, DIM))
    Xe = np.zeros((NK, DIM), np.float32)
    Xe[:N, :] = X
    # xe[c2, p, k, col] = Xe[(c2*20+k)*KT+p, col]  (two contiguous chunks)
    xe_h = np.ascontiguousarray(
        Xe.reshape(2, NKT // 2, KT, DIM).transpose(0, 2, 1, 3)
    ).astype(FP8_NP)

    adjT = np.ascontiguousarray(adj.T)
    deg = adj.sum(axis=1)
    rdeg = (1.0 / np.where(deg > 0, deg, 1.0)).astype(np.float32)

    # per-t weight blocks in the interleaved (8t+d) feature order
    # w1 rows: 0:8 prev-block, 8:16 raw, 16:24 agg  (as in the chain math)
    w1 = np.zeros((24, DIM), np.float32)
    for t in range(T):
        w1[0:7, t * 8:t * 8 + 7] = his_W[t][:, 21:28].T
        w1[7, t * 8 + 7] = cur_W[t][0, 3]
        w1[8:15, t * 8:t * 8 + 7] = his_W[t][:, 0:7].T
        w1[15, t * 8 + 7] = cur_W[t][0, 0]
        w1[16:23, t * 8:t * 8 + 7] = his_W[t][:, 7:14].T
        w1[23, t * 8 + 7] = cur_W[t][0, 1]
    w2 = np.zeros((8, DIM), np.float32)
    for tp in range(T):
        w2[0:7, tp * 8:tp * 8 + 7] = his_weight[:, 7 * tp:7 * tp + 7].T
        w2[7, tp * 8 + 7] = cur_weight[0, tp]
    # interleaved feature (8t+d) -> reference feature (7t+d | 84+t)
    f_ref = np.array([7 * t + d if d < 7 else 84 + t
                      for t in range(T) for d in range(8)])
    wf96 = final_weight[:, f_ref].T  # [96 (8t+d), 96 (out)]
    wf = np.ascontiguousarray(
        wf96.reshape(T, 8, DIM).transpose(1, 0, 2).reshape(8, T * DIM))

    # chain weights at their partition homes
    w1p = np.zeros((40, T * 8), np.float32)
    w1p[0:8, :] = w1[0:8, :]
    w1p[32:40, :] = w1[0:8, :]
    w1ra = np.zeros((48, T * 8), np.float32)
    w1ra[0:16, :] = w1[8:24, :]
    w1ra[32:48, :] = w1[8:24, :]
    w1x = np.ascontiguousarray(w1[16:24, 0:8])
    w2s = np.zeros((40, T * 8), np.float32)
    w2s[0:8, :] = w2
    w2s[32:40, :] = w2
    wcf = np.zeros((40, T * DIM), np.float32)
    wcf[0:8, :] = wf
    wcf[32:40, :] = wf

    in_maps = []
    for c in range(NCORES):
        c0 = c * NPC
        ac = np.zeros((NK, NP), np.float32)
        ac[:N, :NPC] = adjT[:, c0:c0 + NPC]
        # a[h, g, p, j, n] = ac[(g*KG+j)*KT+p, h*NH+n]
        ah = np.ascontiguousarray(
            ac.reshape(NG, KG, KT, 2, NH).transpose(3, 0, 2, 1, 4)
        ).astype(FP8_NP)
        rbc = np.ones((NP,), np.float32)
        rbc[:NPC] = rdeg[c0:c0 + NPC]
        rb = np.ascontiguousarray(
            np.broadcast_to(rbc[None, :], (DIM, NP))).astype(BF16_NP)
        dtc = np.zeros((8, T, NP), np.float32)
        dtc[:, :, :NPC] = data[:, c0:c0 + NPC, :].transpose(2, 0, 1)
        ptc = np.zeros((8, T, NP), np.float32)
        ptc[:, :, :NPC] = pos[:, c0:c0 + NPC, :].transpose(2, 0, 1)
        # [2, 8, T*NH] halves, contiguous per half
        dtc = np.ascontiguousarray(
            dtc.reshape(8, T, 2, NH).transpose(2, 0, 1, 3).reshape(2, 8, T * NH))
        ptc = np.ascontiguousarray(
            ptc.reshape(8, T, 2, NH).transpose(2, 0, 1, 3).reshape(2, 8, T * NH))
        in_maps.append({
            "a": ah, "xe": xe_h, "rb": rb,
            "dt": dtc.astype(BF16_NP), "pt": ptc.astype(BF16_NP),
            "w1p": w1p.astype(BF16_NP), "w1ra": w1ra.astype(BF16_NP),
            "w1x": w1x.astype(BF16_NP),
            "w2s": w2s.astype(BF16_NP), "wcf": wcf.astype(BF16_NP),
        })
    return in_maps


def assemble(results):
    out = np.empty((N, DIM), np.float32)
    for c in range(NCORES):
        out[c * NPC:(c + 1) * NPC, :] = \
            results[c]["out"][:, :NPC].T.astype(np.float32)
    return out


_NC_CACHE = None


def get_nc():
    global _NC_CACHE
    if _NC_CACHE is None:
        _NC_CACHE = build_nc()
    return _NC_CACHE


def run_spmd(in_maps, **kwargs):
    nc = get_nc()
    return bass_utils.run_bass_kernel_spmd(
        nc, in_maps, list(range(NCORES)), **kwargs)


def kernel(**inputs):
    in_maps = prep_in_maps(**inputs)
    res = run_spmd(in_maps)
    return assemble(res.results)
